# revision 24
# baseline (speedup 1.0000x reference)
"""CORAL loss kernel for Trainium2 (8 NeuronCores, Bass/Tile).

Strategy (data-parallel over bz, per sharding hint):
  - Shard features [32, 4096, 256] along bz: 4 batch elements per core.
  - Host casts features to fp8-e4m3 and appends a ones column (d -> d+1):
    the device reads 1/4 of the fp32 bytes, and the PE runs DoubleRow fp8
    matmuls (2 k-rows per instruction, half the instruction stream of the
    fp16 variant). PSUM accumulation stays fp32. fp8 quantization costs
    ~1.0e-3 relative loss error (measured end to end; tolerance is 2e-2 -
    the CORAL loss averages ~33M quantization noises, and the shared
    quantization bias on the cov diagonals cancels in the pairwise
    differences).
  - Per batch element b on device: partition p of SBUF holds 32 consecutive
    rows of xaug[b]. The PE accumulates, in PSUM, ps0 = S rows 0:128 (all
    257 cols: S block plus the colsum column from the ones trick) and
    ps1 = S rows 128:256, cols 128:257 only (S is symmetric; the host
    mirrors the lower block). DVE stages PSUM to SBUF as fp16; one DMA per
    batch writes the packed block out.
  - Host (float64): reassemble S, cov_b = (S_b - colsum_b x m_b)/(n-1),
    then the tiny masked pairwise CORAL reduction (exact mirror of the
    reference math).

Hardware notes (from the fp16 baseline's trace):
  - DMA trigger instructions (DMA_DIRECT2D) cost ~620ns EACH on the issuing
    engine, so x loads alternate between the SP (sync) and Activation
    (scalar) HWDGE rings to double the issue rate; kc=32 means one load per
    batch element. Chunk descriptors stripe across all 16 DMA engines and
    drain FIFO per engine, so chunks complete in issue order at the
    ~360-400 GB/s per-core aggregate.
  - Out-stores also go on the HWDGE rings (they are emitted after every
    load on those FIFO rings, so their DVE-wait blocks nothing), avoiding
    the ~2us SWDGE (gpsimd) descriptor-generation latency on the last
    batch's store, which is on the critical path. Tile round-robins HWDGE
    DMAs over 8 DMAHW completion-sem lanes; 4 loads + 4 stores fit exactly,
    so no DMA ever needs a lane-ordering wait on top of its data wait (the
    HW encodes at most one wait per DMA).
  - Most instructions carry at most ONE semaphore wait: x tiles get
    dedicated SBUF slots (x DMAs never wait), PSUM banks are claimed by a
    tiny const-only matmul pinned (order-only dep) after the bank's
    previous user's PE "fence", and the fence reads the staged output tile
    so the DVE-release of the bank is transitively implied. Tile's
    kernel-tail Drain is split into single-wait drains by a JSON post-pass.
  - The PE clock is HAM-gated (1.2 GHz until ~3.4us of sustained activity):
    warm-up matmuls on a memset constant run while the first x chunks are
    still in flight, so the real matmul stream starts near 2.4 GHz.
"""

import re
import sys

import numpy as np

if "/opt/trn_rl_repo" not in sys.path:
    sys.path.insert(0, "/opt/trn_rl_repo")

import concourse.bass as bass
import concourse.mybir as mybir
import concourse.tile as tile
from concourse.tile_rust import add_dep_helper

BZ, N, D = 32, 4096, 256
NCORES = 8
BPC = BZ // NCORES  # batch elements per core
P = 128  # partitions


DPAD = 272  # row stride in fp8 bytes: 257 data cols + 15 pad (16 | 272)


def build_nc(bpc=BPC, n=N, d=D, kc=32, ps_bufs=3, warmup=4, warmn=384):
    """Per-core Bass module: raw S blocks for `bpc` batch elements.

    Input "x": host-prepared fp8-e4m3 [bpc, n, DPAD] ([X | ones | 0-pad]).
    Rows are padded 257 -> 272 bytes: the dual-fp8 LDWEIGHTS/MATMUL ISA
    requires the 2-row pair dim of the 3D AP to have a 16-byte-multiple
    step (and 16B-aligned bases) - see s3_lw_dual_fp8_restrictions.
    Output "outs": fp16 [bpc, 128, 386] packed per-batch blocks
    [S[0:128, 0:256] | colsum[0:128]] ++ [S[128:256, 128:256] | colsum[128:256]].
    """
    assert n % P == 0 and d == 2 * P
    kt = n // P  # k-tiles of 128 rows
    assert kt % kc == 0 and kc % 2 == 0

    # Per-batch load-chunk splits (in k-tiles). Batch 0 leads with a tiny
    # 4-ktile chunk (time-to-first-completion ~ its wire time, so the PE's
    # real stream starts ~1.3us earlier and the HAM clock ramp is never
    # reset by an idle gap); batches 1-2 load whole (the cold-clock PE runs
    # ~1.5 chunks behind by then, so their coarse completion sems cause no
    # stall); batch 3 splits in half so the PE tail after the final DMA is
    # 8 pairs. Exactly 7 loads: with the last batch's store that fills the
    # 8 DMAHW completion-sem lanes Tile round-robins over, so no HWDGE DMA
    # needs a lane-ordering wait on top of its data wait (the HW encodes at
    # most one wait per DMA). A load's only wait is its lane wait anyway
    # (dedicated SBUF slots).
    def chunk_split(b):
        if b == 0:
            return [4, 12, kc // 2]
        return [kc // 2, kc // 2]

    # One slot per chunk-load: x-tile slots are never reused, so x DMAs
    # never need a slot-release wait (DMAs also carry at most one wait).
    xp_bufs = sum(len(chunk_split(b)) for b in range(bpc))

    nc = bass.Bass(trn_type="TRN2", enable_partition_id=False)
    f32 = mybir.dt.float32
    f16 = mybir.dt.float16
    f8 = mybir.dt.float8e4
    dbl = mybir.MatmulPerfMode.DoubleRow
    x = nc.dram_tensor("x", [bpc, n, DPAD], f8, kind="ExternalInput")
    w0, w1 = d + 1, d // 2 + 1
    # fp16 stats output: halves the write traffic; S diag ~n gives fp16 abs
    # err ~2 -> cov err ~5e-4 per diag entry, which averages out to ~1e-5
    # relative on the loss.
    outs = nc.dram_tensor("outs", [bpc, P, w0 + w1], f16, kind="ExternalOutput")

    with tile.TileContext(nc) as tc:
        with (
            tc.tile_pool(name="xp", bufs=xp_bufs) as xp,
            tc.tile_pool(name="op", bufs=bpc) as op,
            tc.tile_pool(name="constp", bufs=1) as constp,
            tc.tile_pool(name="psp", bufs=ps_bufs, space="PSUM") as psp,
            tc.tile_pool(name="warmp", bufs=1, space="PSUM") as warmp,
        ):
            # Constant operand for warm-up/claim matmuls (DVE memset: cheap,
            # runs during the framework preamble).
            wrm = constp.tile([P, warmn], f16)
            nc.vector.memset(wrm[:, :], 1.0)

            # HAM warm-up: keep the PE busy while the first x chunks are in
            # flight so the clock gate is near 8/8 (2.4 GHz) when the real
            # stream starts.
            wps = warmp.tile([1, warmn], f32)
            for _ in range(warmup):
                nc.tensor.matmul(
                    wps[0:1, :], wrm[:, 0:1], wrm[:, 0:warmn],
                    start=True, stop=True, skip_group_check=True,
                )

            def claim(pstile, after=None):
                # Tiny const-only matmul whose only job is to carry the PSUM
                # bank slot-release wait (one-wait-per-PE-instruction limit).
                # Garbage value; cleared by start=True of the first real use.
                inst = nc.tensor.matmul(
                    pstile[0:1, 0:1], wrm[:, 0:1], wrm[:, 0:1],
                    start=True, stop=True, skip_group_check=True,
                )
                if after is not None:
                    # Pin the claim after the fence of the bank's previous
                    # user (same engine, order-only): the DVE-release wait is
                    # then implied by the fence's wait and elided, leaving
                    # only the PE bank-drain wait.
                    add_dep_helper(inst.ins, after.ins, sync=False,
                                   reason="psum claim after fence")
                return inst

            # Issue ALL x loads up front: each gets a dedicated SBUF slot and
            # has no dependencies. Alternate the SP and Activation HWDGE
            # rings: the ~620ns DMA_DIRECT2D trigger cost is per-engine, so
            # two rings double the issue rate and halve time-to-first-chunk.
            ld_engines = [nc.sync, nc.scalar]
            xts = {}
            li = 0
            for b in range(bpc):
                k0 = 0
                for c, kcc in enumerate(chunk_split(b)):
                    xt = xp.tile([P, kcc, DPAD], f8, tag=f"xt{kcc}",
                                 name=f"xt_{b}_{c}")
                    # Partition p holds consecutive rows -> contiguous DMA.
                    src = x[b].rearrange("(p k) e -> p k e", p=P)[
                        :, k0 : k0 + kcc, :
                    ]
                    ld_engines[li % 2].dma_start(out=xt[:, :, :], in_=src)
                    xts[b, c] = xt
                    li += 1
                    k0 += kcc

            def emit_kloop(b, fence=None, tail_split=0):
                ps0 = psp.tile([P, w0], f32, tag="ps0", name=f"ps0_{b}")
                ps1 = psp.tile([P, w1], f32, tag="ps1", name=f"ps1_{b}")
                claim(ps0, after=fence)
                claim(ps1, after=fence)
                npair = kt // 2
                # With tail_split=T, the last T pairs are emitted as all-ps0
                # then all-ps1 (accumulation order is free): ps0 hits its
                # stop ~T ps1-matmuls early, so its 257-col DVE cast
                # overlaps the end of the matmul stream and only ps1's
                # 129-col cast is on the critical tail.
                mm0s, mm1s = [], []
                j = 0
                for c, kcc in enumerate(chunk_split(b)):
                    xt = xts[b, c]
                    for lk in range(0, kcc, 2):
                        # DoubleRow: one instruction contracts 2 k-rows;
                        # operand dim1 is the 2-row pair ([P, 2, free]).
                        mm0s.append((
                            ps0, xt[:, lk : lk + 2, 0:P],
                            xt[:, lk : lk + 2, 0:w0], j == 0,
                        ))
                        mm1s.append((
                            ps1, xt[:, lk : lk + 2, P:d],
                            xt[:, lk : lk + 2, P : d + 1], j == 0,
                        ))
                        j += 1

                def mm(args, stop):
                    ps, lhsT, rhs, start = args
                    nc.tensor.matmul(ps[:, :], lhsT, rhs, start=start,
                                     stop=stop, perf_mode=dbl)

                head = npair - tail_split
                for j in range(head):
                    mm(mm0s[j], stop=(tail_split == 0 and j == npair - 1))
                    mm(mm1s[j], stop=(tail_split == 0 and j == npair - 1))
                for j in range(head, npair):
                    mm(mm0s[j], stop=(j == npair - 1))
                for j in range(head, npair):
                    mm(mm1s[j], stop=(j == npair - 1))
                return ps0, ps1

            def emit_epilogue(b, ps0, ps1):
                ot = op.tile([P, w0 + w1], f16, tag="ot", name=f"ot_{b}")
                nc.vector.tensor_copy(ot[:, 0:w0], ps0[:, :])
                cp1 = nc.vector.tensor_copy(ot[:, w0 : w0 + w1], ps1[:, :])
                if b == bpc - 1:
                    # The last batch's store (critical path) rides HWDGE -
                    # no SWDGE descriptor-generation latency. Its lane
                    # collides with a load's (9 loads booked all 8 DMAHW
                    # lanes), so the store would need lane-ordering + DVE
                    # waits - two, where the HW encodes one. Bridge: an SP
                    # nop carries the DVE wait; the store's DVE wait is then
                    # implied by SP program order and elided, leaving only
                    # the (long-satisfied) lane wait.
                    nop = nc.sync.nop()
                    add_dep_helper(nop.ins, cp1.ins, sync=True,
                                   reason="carry DVE wait for the store")
                    st = nc.sync.dma_start(out=outs[b], in_=ot[:, :])
                    add_dep_helper(st.ins, nop.ins, sync=False,
                                   reason="store after DVE-wait bridge")
                else:
                    # SWDGE (DMASW lanes); latency hides mid-stream.
                    nc.gpsimd.dma_start(out=outs[b], in_=ot[:, :])
                if b + ps_bufs >= bpc:
                    # No later batch reuses these banks; the kernel-tail
                    # drains cover the ordering. Skip the fence matmul.
                    return None
                # PE fence: reads the region written by the LAST DVE copy,
                # so the PE's observed DVE clock passes both PSUM reads; the
                # next claim of these banks then needs no explicit DVE wait.
                # Writes garbage into ps0 after its data was staged.
                return nc.tensor.matmul(
                    ps0[0:1, 0:1],
                    ot[:, w0 + w1 - 1 : w0 + w1], ot[:, w0 + w1 - 1 : w0 + w1],
                    start=True, stop=True, skip_group_check=True,
                )

            # One-batch software pipeline: epilogue(b) is emitted after
            # kloop(b+1) so the PE stream never stalls on the epilogue.
            prev = None
            fences = {}
            for b in range(bpc):
                cur = emit_kloop(b, fence=fences.get(b - ps_bufs),
                                 tail_split=5 if b == bpc - 1 else 0)
                if prev is not None:
                    fences[b - 1] = emit_epilogue(b - 1, *prev)
                prev = cur
            emit_epilogue(bpc - 1, *prev)

    _install_drain_split(nc)
    return nc


def _split_drain_waits(bir, max_waits=1):
    """Split any Drain carrying more than `max_waits` sem waits into a chain
    of single-wait Drains (the HW sync-wait table is tiny; Tile's kernel-tail
    drain waits on every active sem lane at once)."""
    for fn in bir["functions"]:
        for blk in fn["blocks"]:
            out = []
            changed = False
            for inst in blk["instructions"]:
                waits = (inst.get("sync_info") or {}).get("on_wait") or []
                if inst.get("opcode") == "Drain" and len(waits) > max_waits:
                    changed = True
                    for wi in range(0, len(waits) - max_waits):
                        clone = {
                            **inst,
                            "name": f"{inst['name']}_w{wi}",
                            "sync_info": {
                                "on_wait": [waits[wi]],
                                "on_update": [],
                            },
                        }
                        out.append(clone)
                    inst = {
                        **inst,
                        "sync_info": {
                            **inst["sync_info"],
                            "on_wait": waits[len(waits) - max_waits :],
                        },
                    }
                out.append(inst)
            if changed:
                blk["instructions"] = out
    return bir


def _elide_covered_waits(bir):
    """Drop sem-ge-imm waits already guaranteed by an earlier wait on the
    SAME engine in the same block (engines execute their stream in order and
    Tile's tick semaphores only increase during the kernel). This lets an
    instruction that would need two waits (HW limit: one) ride a preceding
    single-purpose carrier (e.g. the SP nop before the last HWDGE store)."""
    for fn in bir["functions"]:
        for blk in fn["blocks"]:
            seen = {}  # (engine, sem id) -> max waited value
            for inst in blk["instructions"]:
                si = inst.get("sync_info") or {}
                waits = si.get("on_wait") or []
                if not waits:
                    continue
                eng = inst.get("engine")
                kept = []
                for w in waits:
                    name = w.get("ant_name") or ""
                    # Only the engine/DMA tick sems are monotonic during the
                    # kernel; barrier/block sems get range-cleared by the
                    # exit-barrier protocol and must keep their waits.
                    monotonic = bool(
                        re.match(r"^(DMAHW|DMASW)\d+_", name)
                        or re.match(r"^(DVE|PE|SP|Pool|Activation)_", name)
                    )
                    if monotonic and w.get("wait_mode") == "sem-ge-imm":
                        key = (eng, w.get("id"))
                        v = w.get("wait_value", 0)
                        if seen.get(key, -1) >= v:
                            continue  # covered by earlier same-engine wait
                        seen[key] = max(seen.get(key, -1), v)
                    kept.append(w)
                if len(kept) != len(waits):
                    inst["sync_info"] = {**si, "on_wait": kept}
    return bir


def _install_drain_split(nc):
    import orjson

    raw = nc.to_json_bytes

    def patched():
        return orjson.dumps(
            _split_drain_waits(_elide_covered_waits(orjson.loads(raw())))
        )

    nc.to_json_bytes = patched


_NC_CACHE = {}


def _get_nc():
    key = (BPC, N, D)
    if key not in _NC_CACHE:
        _NC_CACHE[key] = build_nc()
    return _NC_CACHE[key]


def augment_ones_f8(feats, bpc, n, d):
    """[cores, bpc, n, d] fp32 -> per-core fp8-e4m3 [bpc, n, DPAD]:
    [X | ones | 0-pad] (rows padded to a 16-byte-multiple stride)."""
    import ml_dtypes

    out = np.zeros((feats.shape[0], bpc, n, DPAD), dtype=ml_dtypes.float8_e4m3)
    out[..., :d] = feats.astype(ml_dtypes.float8_e4m3)
    out[..., d] = 1.0
    return out


def stats_from_raw(outs_blocks, n=N, d=D):
    """Device outs [bz, 128, 386] (packed, see build_nc) -> f64 stats."""
    bz = outs_blocks.shape[0]
    h = d // 2
    o = outs_blocks.astype(np.float64)
    s = np.empty((bz, d, d))
    s[:, :h, :] = o[:, :, 0:d]
    s[:, h:, h:] = o[:, :, d + 1 : d + 1 + h]
    s[:, h:, :h] = np.swapaxes(o[:, :, h:d], 1, 2)  # symmetry mirror
    colsum = np.concatenate([o[:, :, d], o[:, :, d + 1 + h]], axis=1)
    m = colsum / n
    covs = (s - colsum[:, :, None] * m[:, None, :]) / (n - 1)
    return m, covs


def coral_from_stats(means, covs, domains, d=D):
    """Masked pairwise CORAL reduction from per-batch stats (float64)."""
    bz = means.shape[0]
    m = means.astype(np.float64)
    ms = (m * m).sum(1)
    md = (ms[:, None] + ms[None, :] - 2.0 * (m @ m.T)) / d
    v = covs.astype(np.float64).reshape(bz, -1)
    cs = (v * v).sum(1)
    g = v @ v.T
    cd = (cs[:, None] + cs[None, :] - 2.0 * g) / (d * d)
    upper = np.triu(np.ones((bz, bz), dtype=bool), k=1)
    mask = upper & (np.asarray(domains)[:, None] != np.asarray(domains)[None, :])
    loss = np.where(mask, md + cd, 0.0).sum()
    num = int(mask.sum())
    if num > 1:
        loss = loss / num
    return np.float32(loss)


def kernel(features, domains, _trace=False):
    from concourse import bass_utils

    feats = np.asarray(features)
    assert feats.shape == (BZ, N, D)
    xaug = augment_ones_f8(
        np.asarray(feats, dtype=np.float32).reshape(NCORES, BPC, N, D), BPC, N, D
    )
    nc = _get_nc()
    in_maps = [{"x": xaug[c]} for c in range(NCORES)]
    res = bass_utils.run_bass_kernel_spmd(
        nc, in_maps, core_ids=list(range(NCORES)), trace=_trace
    )
    blocks = np.concatenate([r["outs"] for r in res.results], axis=0)
    means, covs = stats_from_raw(blocks)
    out = coral_from_stats(means, covs, domains)
    if _trace:
        return out, res
    return out
